# revision 1
# baseline (speedup 1.0000x reference)
"""Trainium2 Bass kernel for the DiffSSM block.

Strategy: data-parallel over batch B=8 across the 8 NeuronCores (one batch
element per core). All heavy compute (two D x D projections, two kernel-3
convolutions over channels, and the bidirectional SSM global convolution,
expressed as a single L x L Toeplitz matmul with beta1/beta2 folded in) runs
on the TensorEngine in bf16 with fp32 PSUM accumulation. The tiny SSM kernel
generation, the timestep embedding, and the Toeplitz matrix construction are
O(L*N + B*D + L^2) host-side precomputation, replicated across cores.

Device-side dataflow per core (L=2048, D=1024, P=128):
  A: h = x @ Wi + bi (lhsT = xT), LN1 -> h_ln (L-part, D-free) bf16;
     h_ln also written to DRAM scratch.
  C: h_ln DRAM -> SBUF transposed via xbar DMA-transpose -> h_lnT (D-part,
     L-free, zero-padded borders for the conv shifts).
  B: mixT = (T_mix @ h_ln)^T via lhsT = h_ln tiles, rhs = T_mixT chunks;
     h2T = mixT * noise_scale[d] (per-partition scalar in T layout).
  D: conv1 as 3 shifted matmuls accumulated in PSUM, evicted through
     ScalarE Silu(+bc1) -> coT.
  E: conv2 likewise, fused eviction h2T += c2 + bc2.
  F: y = h2 @ Wo + bo (lhsT = h2T tiles), LN2, residual add with x,
     DMA out fp32.
"""

import math

import numpy as np
import ml_dtypes

_BF16 = ml_dtypes.bfloat16

_L, _D, _B = 2048, 1024, 8

_cache = {}


def _build(L, D, n_cores, debug_taps=False):
    import concourse.bacc as bacc
    import concourse.bass as bass
    import concourse.tile as tile
    from concourse import mybir

    f32 = mybir.dt.float32
    bf16 = mybir.dt.bfloat16
    AF = mybir.ActivationFunctionType
    OP = mybir.AluOpType

    P = 128
    KT = D // P            # feature tiles (contraction / d / o / i tiles)
    LT = L // P            # sequence tiles
    ND = min(512, D)       # matmul free-dim chunk along features
    NF = min(512, L)       # matmul free-dim chunk along sequence
    EH = D // ND
    LC = L // NF
    ST = LT

    nc = bacc.Bacc("TRN2", target_bir_lowering=False, debug=False,
                   num_devices=n_cores)

    x_res = nc.dram_tensor("x_res", (L, D), f32, kind="ExternalInput").ap()
    xT = nc.dram_tensor("xT", (D, L), bf16, kind="ExternalInput").ap()
    Wi = nc.dram_tensor("Wi", (D, D), bf16, kind="ExternalInput").ap()
    w1T = nc.dram_tensor("w1T", (KT, P, 3, D), bf16, kind="ExternalInput").ap()
    w2T = nc.dram_tensor("w2T", (KT, P, 3, D), bf16, kind="ExternalInput").ap()
    Wo = nc.dram_tensor("Wo", (D, D), bf16, kind="ExternalInput").ap()
    TmT = nc.dram_tensor("TmT", (L, L), bf16, kind="ExternalInput").ap()
    nsc = nc.dram_tensor("nsc", (P, KT), f32, kind="ExternalInput").ap()
    bc1c = nc.dram_tensor("bc1c", (P, KT), f32, kind="ExternalInput").ap()
    bc2c = nc.dram_tensor("bc2c", (P, KT), f32, kind="ExternalInput").ap()
    vec_names = ["biv", "g1v", "b1v", "g2v", "b2v", "bov"]
    vecs = {n: nc.dram_tensor(n, (D,), f32, kind="ExternalInput").ap()
            for n in vec_names}
    out = nc.dram_tensor("out", (L, D), f32, kind="ExternalOutput").ap()
    taps = {}
    if debug_taps:
        KT_ = D // 128
        taps["hln"] = nc.dram_tensor("tap_hln", (L, D), bf16,
                                     kind="ExternalOutput").ap()
        taps["hlnT"] = nc.dram_tensor("tap_hlnT", (128, KT_, L), bf16,
                                      kind="ExternalOutput").ap()
        taps["mix"] = nc.dram_tensor("tap_mix", (128, KT_, L), bf16,
                                     kind="ExternalOutput").ap()
        taps["co"] = nc.dram_tensor("tap_co", (128, KT_, L), bf16,
                                    kind="ExternalOutput").ap()
        taps["h2T"] = nc.dram_tensor("tap_h2T", (128, KT_, L), bf16,
                                     kind="ExternalOutput").ap()
        taps["wo"] = nc.dram_tensor("tap_wo", (128, KT_, D), bf16,
                                    kind="ExternalOutput").ap()
        taps["y"] = nc.dram_tensor("tap_y", (L, D), f32,
                                   kind="ExternalOutput").ap()
        taps["yln"] = nc.dram_tensor("tap_yln", (L, D), f32,
                                     kind="ExternalOutput").ap()
        taps["g2"] = nc.dram_tensor("tap_g2", (128, D), f32,
                                    kind="ExternalOutput").ap()
        taps["mv2"] = nc.dram_tensor("tap_mv2", (L // 128, 128, 2), f32,
                                     kind="ExternalOutput").ap()
        taps["fin"] = nc.dram_tensor("tap_fin", (L, D), f32,
                                     kind="ExternalOutput").ap()

    bn_fmax = math.gcd(512, D)
    n_sub = D // bn_fmax

    with tile.TileContext(nc) as tc:
        const = tc.alloc_tile_pool(name="const", bufs=1)
        psum = tc.alloc_tile_pool(name="psum", bufs=6, space="PSUM")
        statp = tc.alloc_tile_pool(name="stat", bufs=4)
        hbufp = tc.alloc_tile_pool(name="hbuf", bufs=3)
        dramp = tc.alloc_tile_pool(name="drams", bufs=1, space="DRAM")

        rep = {}
        for n in vec_names:
            t = const.tile([P, D], f32, tag=n, name=f"rep_{n}")
            ap = vecs[n]
            bcast = bass.AP(tensor=ap.tensor, offset=ap.offset,
                            ap=[[0, P]] + list(ap.ap))
            nc.gpsimd.dma_start(out=t[:], in_=bcast)
            rep[n] = t
        ns_sb = const.tile([P, KT], f32)
        nc.sync.dma_start(out=ns_sb[:], in_=nsc)
        bc1_sb = const.tile([P, KT], f32)
        nc.sync.dma_start(out=bc1_sb[:], in_=bc1c)
        bc2_sb = const.tile([P, KT], f32)
        nc.sync.dma_start(out=bc2_sb[:], in_=bc2c)
        eps_sb = const.tile([P, 1], f32)
        nc.vector.memset(eps_sb[:], 1e-5)

        h2T_pool = tc.alloc_tile_pool(name="h2T", bufs=1)
        h2T_sb = h2T_pool.tile([P, KT, L], bf16)
        hln_pool = tc.alloc_tile_pool(name="hln", bufs=1, side="right")
        hln_sb = hln_pool.tile([P, LT, D], bf16)
        hln_dram = dramp.tile([L, D], bf16)

        # ---- Phase A: proj-in + LN1 ----
        pa_pool = tc.alloc_tile_pool(name="pa", bufs=1)
        xT_sb = pa_pool.tile([P, KT, L], bf16)
        wi_sb = pa_pool.tile([P, KT, D], bf16)
        xT_r = xT.rearrange("(kt p) l -> kt p l", p=P)
        wi_r = Wi.rearrange("(kt p) d -> kt p d", p=P)
        for kt in range(KT):
            nc.sync.dma_start(out=xT_sb[:, kt, :], in_=xT_r[kt])
            nc.sync.dma_start(out=wi_sb[:, kt, :], in_=wi_r[kt])
        hd_r = hln_dram[:].rearrange("(t p) d -> t p d", p=P)

        def layer_norm(buf, g_rep, b_rep, out_ap, tap_mv=None):
            stats = statp.tile([P, n_sub, 6], f32, tag="stats", name="stats")
            for s in range(n_sub):
                nc.vector.bn_stats(out=stats[:, s, :],
                                   in_=buf[:, s * bn_fmax:(s + 1) * bn_fmax])
            mv = statp.tile([P, 2], f32, tag="mv", name="mv")
            nc.vector.bn_aggr(out=mv[:], in_=stats[:])
            if tap_mv is not None:
                nc.gpsimd.dma_start(out=tap_mv, in_=mv[:])
            rstd = statp.tile([P, 1], f32, tag="rstd", name="rstd")
            nc.scalar.activation(out=rstd[:], in_=mv[:, 1:2], func=AF.Sqrt,
                                 bias=eps_sb[:], scale=1.0)
            nc.vector.reciprocal(out=rstd[:], in_=rstd[:])
            nc.vector.tensor_scalar(out=buf[:], in0=buf[:], scalar1=mv[:, 0:1],
                                    scalar2=rstd[:], op0=OP.subtract,
                                    op1=OP.mult)
            nc.vector.tensor_mul(out=buf[:], in0=buf[:], in1=g_rep[:])
            nc.vector.tensor_add(out=out_ap, in0=buf[:], in1=b_rep[:])

        for lt in range(LT):
            h_f32 = pa_pool.tile([P, D], f32, tag="h_f32", name="h_f32",
                                 bufs=3)
            for eh in range(EH):
                ps = psum.tile([P, ND], f32, tag="ps", name="ps")
                for kt in range(KT):
                    nc.tensor.matmul(ps[:],
                                     lhsT=xT_sb[:, kt, lt * P:(lt + 1) * P],
                                     rhs=wi_sb[:, kt, eh * ND:(eh + 1) * ND],
                                     start=(kt == 0), stop=(kt == KT - 1))
                nc.vector.tensor_add(out=h_f32[:, eh * ND:(eh + 1) * ND],
                                     in0=ps[:],
                                     in1=rep["biv"][:, eh * ND:(eh + 1) * ND])
            layer_norm(h_f32, rep["g1v"], rep["b1v"], hln_sb[:, lt, :])
            nc.scalar.dma_start(out=hd_r[lt], in_=hln_sb[:, lt, :])
        pa_pool.release()
        if debug_taps:
            tap_r = taps["hln"].rearrange("(t p) d -> t p d", p=P)
            for lt in range(LT):
                nc.sync.dma_start(out=tap_r[lt], in_=hln_sb[:, lt, :])

        # ---- Phase C: transposed reload (xbar) ----
        # Xbar transpose into a fully contiguous tile at offset 0 (the only
        # destination shape validated on hardware). Conv border columns are
        # handled by narrowing the edge matmuls instead of zero padding.
        hlnT_pool = tc.alloc_tile_pool(name="hlnT", bufs=1)
        hlnT_sb = hlnT_pool.tile([P, KT, L], bf16)
        nc.scalar.dma_start_transpose(out=hlnT_sb[:], in_=hln_dram[:])

        if debug_taps:
            nc.sync.dma_start(out=taps["hlnT"], in_=hlnT_sb[:])

        # ---- Phase B: SSM Toeplitz mix ----
        tb_pool = tc.alloc_tile_pool(name="tb", bufs=2)
        Tm_r = TmT.rearrange("(st p) t -> p st t", p=P)
        for tch in range(LC):
            Tc_sb = tb_pool.tile([P, ST, NF], bf16, tag="Tc", name="Tc")
            for st in range(ST):
                nc.sync.dma_start(out=Tc_sb[:, st, :],
                                  in_=Tm_r[:, st, tch * NF:(tch + 1) * NF])
            for dt in range(KT):
                ps = psum.tile([P, NF], f32, tag="ps", name="ps")
                for st in range(ST):
                    nc.tensor.matmul(ps[:],
                                     lhsT=hln_sb[:, st, dt * P:(dt + 1) * P],
                                     rhs=Tc_sb[:, st, :],
                                     start=(st == 0), stop=(st == ST - 1))
                nc.vector.tensor_scalar_mul(
                    out=h2T_sb[:, dt, tch * NF:(tch + 1) * NF],
                    in0=ps[:], scalar1=ns_sb[:, dt:dt + 1])
        tb_pool.release()
        hln_pool.release()
        if debug_taps:
            nc.sync.dma_start(out=taps["mix"], in_=h2T_sb[:])

        # ---- Phase D: conv1 (+Silu) ----
        w1_pool = tc.alloc_tile_pool(name="w1", bufs=1)
        w1_sb = w1_pool.tile([P, KT, 3, D], bf16)
        for it in range(KT):
            nc.sync.dma_start(out=w1_sb[:, it, :, :], in_=w1T[it])
        co_pool = tc.alloc_tile_pool(name="co", bufs=1, side="right")
        co_sb = co_pool.tile([P, KT, L], bf16)

        def conv_mms(ps, w_sb, src_sb, ot, lc):
            # kernel-3 conv as 3 shifted matmuls; j=1 (no shift, full width)
            # goes first so start=True initializes the whole PSUM range, and
            # the zero-pad border columns are simply skipped.
            first = True
            for it in range(KT):
                for j in (1, 0, 2):
                    o0 = 1 if (j == 0 and lc == 0) else 0
                    o1 = NF - 1 if (j == 2 and lc == LC - 1) else NF
                    base = lc * NF + j - 1
                    nc.tensor.matmul(
                        ps[:, o0:o1],
                        lhsT=w_sb[:, it, j, ot * P:(ot + 1) * P],
                        rhs=src_sb[:, it, base + o0:base + o1],
                        start=first,
                        stop=(it == KT - 1 and j == 2))
                    first = False
        for lc in range(LC):
            for ot in range(KT):
                ps = psum.tile([P, NF], f32, tag="ps", name="ps")
                conv_mms(ps, w1_sb, hlnT_sb, ot, lc)
                nc.scalar.activation(
                    out=co_sb[:, ot, lc * NF:(lc + 1) * NF],
                    in_=ps[:], func=AF.Silu, bias=bc1_sb[:, ot:ot + 1],
                    scale=1.0)
        if debug_taps:
            nc.sync.dma_start(out=taps["co"], in_=co_sb[:])
        w1_pool.release()
        hlnT_pool.release()


        # ---- Phase E: conv2, accumulate into h2T ----
        w2_pool = tc.alloc_tile_pool(name="w2", bufs=1)
        w2_sb = w2_pool.tile([P, KT, 3, D], bf16)
        for it in range(KT):
            nc.sync.dma_start(out=w2_sb[:, it, :, :], in_=w2T[it])
        for lc in range(LC):
            for ot in range(KT):
                ps = psum.tile([P, NF], f32, tag="ps", name="ps")
                conv_mms(ps, w2_sb, co_sb, ot, lc)
                nc.vector.scalar_tensor_tensor(
                    out=h2T_sb[:, ot, lc * NF:(lc + 1) * NF],
                    in0=ps[:], scalar=bc2_sb[:, ot:ot + 1],
                    in1=h2T_sb[:, ot, lc * NF:(lc + 1) * NF],
                    op0=OP.add, op1=OP.add)
        w2_pool.release()
        co_pool.release()
        if debug_taps:
            nc.sync.dma_start(out=taps["h2T"], in_=h2T_sb[:])

        # ---- Phase F: proj-out + LN2 + residual ----
        wo_pool = tc.alloc_tile_pool(name="wo", bufs=1)
        wo_sb = wo_pool.tile([P, KT, D], bf16)
        wo_r = Wo.rearrange("(dt p) e -> dt p e", p=P)
        for dt in range(KT):
            nc.sync.dma_start(out=wo_sb[:, dt, :], in_=wo_r[dt])
        if debug_taps:
            nc.sync.dma_start(out=taps["wo"], in_=wo_sb[:])
            nc.sync.dma_start(out=taps["g2"], in_=rep["g2v"][:])
            tap_y_r = taps["y"].rearrange("(t p) d -> t p d", p=P)
            tap_yln_r = taps["yln"].rearrange("(t p) d -> t p d", p=P)
            tap_fin_r = taps["fin"].rearrange("(t p) d -> t p d", p=P)
        x_r = x_res.rearrange("(t p) d -> t p d", p=P)
        out_r = out.rearrange("(t p) d -> t p d", p=P)
        for lt in range(LT):
            x_t = hbufp.tile([P, D], f32, tag="x_t", name="x_t", bufs=2)
            nc.sync.dma_start(out=x_t[:], in_=x_r[lt])
            y = hbufp.tile([P, D], f32, tag="y", name="y", bufs=2)
            for eh in range(EH):
                ps = psum.tile([P, ND], f32, tag="ps", name="ps")
                for dt in range(KT):
                    nc.tensor.matmul(ps[:],
                                     lhsT=h2T_sb[:, dt, lt * P:(lt + 1) * P],
                                     rhs=wo_sb[:, dt, eh * ND:(eh + 1) * ND],
                                     start=(dt == 0), stop=(dt == KT - 1))
                nc.vector.tensor_add(out=y[:, eh * ND:(eh + 1) * ND],
                                     in0=ps[:],
                                     in1=rep["bov"][:, eh * ND:(eh + 1) * ND])
            if debug_taps:
                nc.sync.dma_start(out=tap_y_r[lt], in_=y[:])
            layer_norm(y, rep["g2v"], rep["b2v"], y[:],
                       tap_mv=(taps["mv2"][lt] if debug_taps else None))
            if debug_taps:
                nc.sync.dma_start(out=tap_yln_r[lt], in_=y[:])
            out_t = hbufp.tile([P, D], f32, tag="out_t", name="out_t",
                               bufs=2)
            nc.vector.tensor_add(out=out_t[:], in0=y[:], in1=x_t[:])
            nc.sync.dma_start(out=out_r[lt], in_=out_t[:])
            if debug_taps:
                nc.sync.dma_start(out=tap_fin_r[lt], in_=out_t[:])
        wo_pool.release()
        h2T_pool.release()
        dramp.release()
        hbufp.release()
        statp.release()
        psum.release()
        const.release()

    nc.compile()
    return nc


def _bf(a):
    return np.ascontiguousarray(np.asarray(a, np.float32)).astype(_BF16)


def _prep_maps(inputs, L, D, n_cores):
    P = 128
    KT = D // P
    f32 = np.float32
    x = np.asarray(inputs["x"], f32)
    t = np.asarray(inputs["t"], f32)
    beta1 = float(np.asarray(inputs["beta1"], f32)[0])
    beta2 = float(np.asarray(inputs["beta2"], f32)[0])

    # SSM kernels -> mixed Toeplitz (transposed), host fp32
    af = np.diagonal(np.asarray(inputs["Af"], f32))
    ab = np.diagonal(np.asarray(inputs["Ab"], f32))
    l_ar = np.arange(L, dtype=f32)[:, None]
    kf = np.exp(l_ar * af[None, :]) @ (
        np.asarray(inputs["Bf"], f32)[:, 0] * np.asarray(inputs["Cf"], f32)[0]
    ) + np.asarray(inputs["Df"], f32)[0]
    kb = np.exp(l_ar * ab[None, :]) @ (
        np.asarray(inputs["Bb"], f32)[:, 0] * np.asarray(inputs["Cb"], f32)[0]
    ) + np.asarray(inputs["Db"], f32)[0]
    tms = np.arange(L)[None, :] - np.arange(L)[:, None]   # T_mixT[s,t] : t-s
    TmT = (np.where(tms >= 0, beta1 * kf[np.clip(tms, 0, None)], 0.0)
           + np.where(tms <= 0, beta2 * kb[np.clip(-tms, 0, None)], 0.0))
    TmT_bf = TmT.astype(f32).astype(_BF16)

    # timestep embedding -> noise scale (B, D)
    half = D // 2
    freqs = np.exp(np.arange(half, dtype=f32)
                   * (-math.log(10000.0) / (half - 1)))
    ang = t[:, None] * freqs[None, :]
    emb = np.concatenate([np.sin(ang), np.cos(ang)], axis=1).astype(f32)
    ns = (1.0 / (1.0 + np.exp(-emb))).astype(f32)         # (B, D)

    Wi_bf = _bf(inputs["Wi"])
    Wo_bf = _bf(inputs["Wo"])
    w1 = np.asarray(inputs["w1"], f32)
    w2 = np.asarray(inputs["w2"], f32)
    w1T = np.ascontiguousarray(np.transpose(w1, (1, 2, 0))).reshape(
        KT, P, 3, D).astype(_BF16)
    w2T = np.ascontiguousarray(np.transpose(w2, (1, 2, 0))).reshape(
        KT, P, 3, D).astype(_BF16)

    def col(v):
        return np.ascontiguousarray(
            np.asarray(v, f32).reshape(KT, P).T)

    shared = {
        "Wi": Wi_bf, "Wo": Wo_bf, "w1T": w1T, "w2T": w2T, "TmT": TmT_bf,
        "bc1c": col(inputs["bc1"]), "bc2c": col(inputs["bc2"]),
        "biv": np.ascontiguousarray(np.asarray(inputs["bi"], f32)),
        "g1v": np.ascontiguousarray(np.asarray(inputs["g1"], f32)),
        "b1v": np.ascontiguousarray(np.asarray(inputs["b1"], f32)),
        "g2v": np.ascontiguousarray(np.asarray(inputs["g2"], f32)),
        "b2v": np.ascontiguousarray(np.asarray(inputs["b2"], f32)),
        "bov": np.ascontiguousarray(np.asarray(inputs["bo"], f32)),
    }
    in_maps = []
    for b in range(n_cores):
        xb = np.ascontiguousarray(x[b])
        m = dict(shared)
        m["x_res"] = xb
        m["xT"] = np.ascontiguousarray(xb.T.astype(_BF16))
        m["nsc"] = np.ascontiguousarray(ns[b].reshape(KT, P).T)
        in_maps.append(m)
    return in_maps


def get_nc(L=_L, D=_D, n_cores=_B, debug_taps=False):
    key = (L, D, n_cores, debug_taps)
    if key not in _cache:
        _cache[key] = _build(L, D, n_cores, debug_taps)
    return _cache[key]


def kernel(**inputs):
    from concourse.bass_utils import run_bass_kernel_spmd

    L, D, B = _L, _D, _B
    nc = get_nc(L, D, B)
    in_maps = _prep_maps(inputs, L, D, B)
    res = run_bass_kernel_spmd(nc, in_maps, core_ids=list(range(B)))
    return np.stack([res.results[c]["out"] for c in range(B)]).astype(
        np.float32)



# revision 5
# speedup vs baseline: 2.1991x; 2.1991x over previous
"""Trainium2 Bass kernel for the DiffSSM block.

Data-parallel over batch B=8 across 8 NeuronCores (one batch element per
core). All heavy compute runs on the TensorEngine in bf16 with fp32 PSUM
accumulation; the tiny SSM kernel generation, timestep embedding, and
Toeplitz construction are host-side precompute.

v2 engine-balance redesign (vs the phase-serial baseline):
  - Bias adds folded into the matmul accumulation groups as K=1 matmuls
    (ones x bias_row), so LN stats run directly on PSUM.
  - LN normalize (x*rstd - mean*rstd) moved to ScalarE activation with
    per-partition scale/bias APs; only the g/b affine stays on VectorE.
  - Phase B (Toeplitz mix) evicts through ScalarE (Copy, scale=noise),
    phase E eviction stays on VectorE (scalar_tensor_tensor accumulate).
  - hln -> hlnT transpose done as 32 strip-wise xbar DMA transposes
    (per 512-row strip x 128-col block), overlapped with phase A.
  - Conv loops run ot-outer with double-buffered per-ot weight chunks
    (12 KB resident instead of 48 KB), double-buffered TmT chunks, and
    strip-buffered xT loads, so every phase's operands prefetch during
    the previous phase within the SBUF budget.

Device phases: A proj-in+LN1 -> B Toeplitz mix -> D conv1+Silu ->
E conv2 accumulate -> F proj-out+LN2+residual.
"""

import math

import numpy as np
import ml_dtypes

_BF16 = ml_dtypes.bfloat16

_L, _D, _B = 2048, 1024, 8

_cache = {}


def _build(L, D, n_cores):
    import concourse.bacc as bacc
    import concourse.bass as bass
    import concourse.tile as tile
    from concourse import mybir

    f32 = mybir.dt.float32
    bf16 = mybir.dt.bfloat16
    AF = mybir.ActivationFunctionType
    OP = mybir.AluOpType

    P = 128
    KT = D // P            # feature tiles
    LT = L // P            # sequence tiles
    ND = min(512, D)       # matmul free-dim chunk along features
    NF = min(512, L)       # matmul free-dim chunk along sequence
    EH = D // ND
    LC = L // NF
    ST = LT
    SPL = LT // LC         # lt tiles per L-strip (4)

    nc = bacc.Bacc("TRN2", target_bir_lowering=False, debug=False,
                   num_devices=n_cores)

    x_res = nc.dram_tensor("x_res", (L, D), f32, kind="ExternalInput").ap()
    xT = nc.dram_tensor("xT", (D, L), bf16, kind="ExternalInput").ap()
    Wi = nc.dram_tensor("Wi", (D, D), bf16, kind="ExternalInput").ap()
    w1R = nc.dram_tensor("w1R", (KT, P, KT, 3, P), bf16,
                         kind="ExternalInput").ap()
    w2R = nc.dram_tensor("w2R", (KT, P, KT, 3, P), bf16,
                         kind="ExternalInput").ap()
    Wo = nc.dram_tensor("Wo", (D, D), bf16, kind="ExternalInput").ap()
    TmT = nc.dram_tensor("TmT", (L, L), bf16, kind="ExternalInput").ap()
    nsc = nc.dram_tensor("nsc", (P, KT), f32, kind="ExternalInput").ap()
    bc1c = nc.dram_tensor("bc1c", (P, KT), f32, kind="ExternalInput").ap()
    bc2c = nc.dram_tensor("bc2c", (P, KT), f32, kind="ExternalInput").ap()
    bi_row = nc.dram_tensor("bi_row", (1, D), bf16, kind="ExternalInput").ap()
    bo_row = nc.dram_tensor("bo_row", (1, D), bf16, kind="ExternalInput").ap()
    vec_names = ["g1v", "b1v", "g2v"]
    vecs = {n: nc.dram_tensor(n, (D,), f32, kind="ExternalInput").ap()
            for n in vec_names}
    out = nc.dram_tensor("out", (L, D), f32, kind="ExternalOutput").ap()

    with tile.TileContext(nc) as tc:
        # ---- pools (left stack, release order = reverse alloc) ----
        const = tc.alloc_tile_pool(name="const", bufs=1)
        statp = tc.alloc_tile_pool(name="stat", bufs=4)
        psum = tc.alloc_tile_pool(name="psum", bufs=8, space="PSUM")
        dramp = tc.alloc_tile_pool(name="drams", bufs=1, space="DRAM")
        h2T_pool = tc.alloc_tile_pool(name="h2T", bufs=1)
        w2ch_pool = tc.alloc_tile_pool(name="w2ch", bufs=2)
        hlnT_pool = tc.alloc_tile_pool(name="hlnT", bufs=1)
        w1ch_pool = tc.alloc_tile_pool(name="w1ch", bufs=2)
        tb_pool = tc.alloc_tile_pool(name="tb", bufs=2)
        pa_pool = tc.alloc_tile_pool(name="pa", bufs=1)
        # right stack: hln (released end of B), then Fpool, co
        hln_pool = tc.alloc_tile_pool(name="hln", bufs=1, side="right")

        # ---- constants ----
        def rep_tile(name):
            t = const.tile([P, D], f32, tag=name, name=f"rep_{name}")
            ap = vecs[name]
            bcast = bass.AP(tensor=ap.tensor, offset=ap.offset,
                            ap=[[0, P]] + list(ap.ap))
            nc.gpsimd.dma_start(out=t[:], in_=bcast)
            return t

        g1_rep = rep_tile("g1v")
        b1_rep = rep_tile("b1v")
        ns_sb = const.tile([P, KT], f32)
        nc.sync.dma_start(out=ns_sb[:], in_=nsc)
        bc1_sb = const.tile([P, KT], f32)
        nc.sync.dma_start(out=bc1_sb[:], in_=bc1c)
        bc2_sb = const.tile([P, KT], f32)
        nc.sync.dma_start(out=bc2_sb[:], in_=bc2c)
        eps_sb = const.tile([P, 1], f32)
        nc.vector.memset(eps_sb[:], 1e-5)
        ones_sb = const.tile([1, P], bf16)
        nc.vector.memset(ones_sb[:], 1.0)
        bi_sb = const.tile([1, D], bf16)
        nc.sync.dma_start(out=bi_sb[:], in_=bi_row)
        bo_sb = const.tile([1, D], bf16)
        nc.sync.dma_start(out=bo_sb[:], in_=bo_row)

        h2T_sb = h2T_pool.tile([P, KT, L], bf16)
        hlnT_sb = hlnT_pool.tile([P, KT, L], bf16)
        hln_sb = hln_pool.tile([P, LT, D], bf16)
        hln_dram = dramp.tile([L, D], bf16)

        wi_sb = pa_pool.tile([P, KT, D], bf16, tag="wi")
        wi_r = Wi.rearrange("(kt p) d -> kt p d", p=P)
        for kt in range(KT):
            nc.sync.dma_start(out=wi_sb[:, kt, :], in_=wi_r[kt])
        xT_r = xT.rearrange("(kt p) l -> kt p l", p=P)
        hd_r = hln_dram[:].rearrange("(t p) d -> t p d", p=P)
        hdram_flat = hln_dram[:]
        Tm_r = TmT.rearrange("(st p) t -> p st t", p=P)

        def ln_scalars(stats_tile):
            """stats -> (mv, rstd) tiles."""
            mv = statp.tile([P, 2], f32, tag="mv", name="mv")
            nc.vector.bn_aggr(out=mv[:], in_=stats_tile[:])
            std = statp.tile([P, 1], f32, tag="std", name="std")
            nc.scalar.activation(out=std[:], in_=mv[:, 1:2], func=AF.Sqrt,
                                 bias=eps_sb[:], scale=1.0)
            rstd = statp.tile([P, 1], f32, tag="rstd", name="rstd")
            nc.vector.reciprocal(out=rstd[:], in_=std[:])
            return mv, rstd

        # ---- Phase A: proj-in + LN1 (stats on PSUM, norm on ScalarE) ----
        Tc_next = None
        xs = None
        for lt in range(LT):
            ls = lt // SPL
            if lt % SPL == 0:
                xs = pa_pool.tile([P, KT, NF], bf16, tag="xs", name="xs",
                                  bufs=2)
                for kt in range(KT):
                    nc.sync.dma_start(
                        out=xs[:, kt, :],
                        in_=xT_r[kt, :, ls * NF:(ls + 1) * NF])
            col = (lt % SPL) * P
            stats = statp.tile([P, EH, 6], f32, tag="stats", name="stats")
            nrm = statp.tile([P, D], f32, tag="nrm", name="nrm", bufs=3)
            for eh in range(EH):
                ps = psum.tile([P, ND], f32, tag="ps", name="ps")
                nc.tensor.matmul(ps[:], lhsT=ones_sb[:],
                                 rhs=bi_sb[:, eh * ND:(eh + 1) * ND],
                                 start=True, stop=False)
                for kt in range(KT):
                    nc.tensor.matmul(ps[:],
                                     lhsT=xs[:, kt, col:col + P],
                                     rhs=wi_sb[:, kt, eh * ND:(eh + 1) * ND],
                                     start=False, stop=(kt == KT - 1))
                nc.scalar.activation(out=nrm[:, eh * ND:(eh + 1) * ND],
                                     in_=ps[:], func=AF.Copy)
                nc.vector.bn_stats(out=stats[:, eh, :],
                                   in_=nrm[:, eh * ND:(eh + 1) * ND])
            mv, rstd = ln_scalars(stats)
            nc.vector.tensor_scalar(out=nrm[:], in0=nrm[:],
                                    scalar1=mv[:, 0:1], scalar2=rstd[:],
                                    op0=OP.subtract, op1=OP.mult)
            nc.gpsimd.tensor_mul(out=nrm[:], in0=nrm[:], in1=g1_rep[:])
            nc.vector.tensor_add(out=hln_sb[:, lt, :], in0=nrm[:],
                                 in1=b1_rep[:])
            nc.sync.dma_start(out=hd_r[lt], in_=hln_sb[:, lt, :])
            if lt % SPL == SPL - 1:
                # strip-wise xbar transpose: (NF x P) -> (P x NF) per kt
                for kt in range(KT):
                    nc.scalar.dma_start_transpose(
                        out=hlnT_sb[:, kt, ls * NF:(ls + 1) * NF],
                        in_=hdram_flat[ls * NF:(ls + 1) * NF,
                                       kt * P:(kt + 1) * P])
            if lt == 0:
                # prefetch first TmT chunk during A
                Tc_next = tb_pool.tile([P, ST, NF], bf16, tag="Tc",
                                       name="Tc")
                for st in range(ST):
                    nc.sync.dma_start(out=Tc_next[:, st, :],
                                      in_=Tm_r[:, st, 0:NF])
        pa_pool.release()

        # ---- Phase B: SSM Toeplitz mix, evict via ScalarE * noise ----
        w1_next = None
        for tch in range(LC):
            Tc = Tc_next
            if tch + 1 < LC:
                Tc_next = tb_pool.tile([P, ST, NF], bf16, tag="Tc",
                                       name="Tc")
                for st in range(ST):
                    nc.sync.dma_start(
                        out=Tc_next[:, st, :],
                        in_=Tm_r[:, st, (tch + 1) * NF:(tch + 2) * NF])
            if tch == 0:
                # prefetch first conv1 weight chunk during B
                w1_next = w1ch_pool.tile([P, KT, 3, P], bf16, tag="w1c",
                                         name="w1c")
                nc.sync.dma_start(out=w1_next[:], in_=w1R[0])
            for dt in range(KT):
                ps = psum.tile([P, NF], f32, tag="ps", name="ps")
                for st in range(ST):
                    nc.tensor.matmul(ps[:],
                                     lhsT=hln_sb[:, st, dt * P:(dt + 1) * P],
                                     rhs=Tc[:, st, :],
                                     start=(st == 0), stop=(st == ST - 1))
                nc.scalar.activation(
                    out=h2T_sb[:, dt, tch * NF:(tch + 1) * NF],
                    in_=ps[:], func=AF.Copy, scale=ns_sb[:, dt:dt + 1])
        tb_pool.release()
        hln_pool.release()

        # Fpool + co on the (now empty) right stack; loads overlap D/E.
        Fpool = tc.alloc_tile_pool(name="Fp", bufs=1, side="right")
        co_pool = tc.alloc_tile_pool(name="co", bufs=1, side="right")
        wo_sb = Fpool.tile([P, KT, D], bf16, tag="wo")
        wo_r = Wo.rearrange("(dt p) e -> dt p e", p=P)
        for dt in range(KT):
            nc.sync.dma_start(out=wo_sb[:, dt, :], in_=wo_r[dt])
        g2_rep = Fpool.tile([P, D], f32, tag="g2")
        ap = vecs["g2v"]
        nc.gpsimd.dma_start(out=g2_rep[:], in_=bass.AP(
            tensor=ap.tensor, offset=ap.offset, ap=[[0, P]] + list(ap.ap)))
        co_sb = co_pool.tile([P, KT, L], bf16)

        def conv_mms(ps, w_t, src_sb, lc):
            # kernel-3 conv as 3 shifted matmuls; j=1 (no shift) first so
            # start=True initializes the whole PSUM range; border columns
            # handled by narrowing the edge matmuls.
            first = True
            for it in range(KT):
                for j in (1, 0, 2):
                    o0 = 1 if (j == 0 and lc == 0) else 0
                    o1 = NF - 1 if (j == 2 and lc == LC - 1) else NF
                    base = lc * NF + j - 1
                    nc.tensor.matmul(
                        ps[:, o0:o1],
                        lhsT=w_t[:, it, j, :],
                        rhs=src_sb[:, it, base + o0:base + o1],
                        start=first,
                        stop=(it == KT - 1 and j == 2))
                    first = False

        # ---- Phase D: conv1 (+Silu), ot-outer with weight streaming ----
        w2_next = None
        for ot in range(KT):
            w1t = w1_next
            if ot + 1 < KT:
                w1_next = w1ch_pool.tile([P, KT, 3, P], bf16, tag="w1c",
                                         name="w1c")
                nc.sync.dma_start(out=w1_next[:], in_=w1R[ot + 1])
            if ot == 0:
                w2_next = w2ch_pool.tile([P, KT, 3, P], bf16, tag="w2c",
                                         name="w2c")
                nc.sync.dma_start(out=w2_next[:], in_=w2R[0])
            for lc in range(LC):
                ps = psum.tile([P, NF], f32, tag="ps", name="ps")
                conv_mms(ps, w1t, hlnT_sb, lc)
                nc.scalar.activation(
                    out=co_sb[:, ot, lc * NF:(lc + 1) * NF],
                    in_=ps[:], func=AF.Silu, bias=bc1_sb[:, ot:ot + 1],
                    scale=1.0)
        w1ch_pool.release()
        hlnT_pool.release()

        # ---- Phase E: conv2, accumulate into h2T ----
        for ot in range(KT):
            w2t = w2_next
            if ot + 1 < KT:
                w2_next = w2ch_pool.tile([P, KT, 3, P], bf16, tag="w2c",
                                         name="w2c")
                nc.sync.dma_start(out=w2_next[:], in_=w2R[ot + 1])
            for lc in range(LC):
                ps = psum.tile([P, NF], f32, tag="ps", name="ps")
                conv_mms(ps, w2t, co_sb, lc)
                nc.vector.scalar_tensor_tensor(
                    out=h2T_sb[:, ot, lc * NF:(lc + 1) * NF],
                    in0=ps[:], scalar=bc2_sb[:, ot:ot + 1],
                    in1=h2T_sb[:, ot, lc * NF:(lc + 1) * NF],
                    op0=OP.add, op1=OP.add)
        w2ch_pool.release()
        co_pool.release()

        # ---- Phase F: proj-out + LN2 + residual ----
        x_r = x_res.rearrange("(t p) d -> t p d", p=P)
        out_r = out.rearrange("(t p) d -> t p d", p=P)
        for lt in range(LT):
            x_t = Fpool.tile([P, D], f32, tag="x_t", name="x_t", bufs=2)
            nc.sync.dma_start(out=x_t[:], in_=x_r[lt])
            stats = statp.tile([P, EH, 6], f32, tag="stats", name="stats")
            y = Fpool.tile([P, D], f32, tag="y", name="y", bufs=2)
            for eh in range(EH):
                ps = psum.tile([P, ND], f32, tag="ps", name="ps")
                nc.tensor.matmul(ps[:], lhsT=ones_sb[:],
                                 rhs=bo_sb[:, eh * ND:(eh + 1) * ND],
                                 start=True, stop=False)
                for dt in range(KT):
                    nc.tensor.matmul(ps[:],
                                     lhsT=h2T_sb[:, dt, lt * P:(lt + 1) * P],
                                     rhs=wo_sb[:, dt, eh * ND:(eh + 1) * ND],
                                     start=False, stop=(dt == KT - 1))
                nc.scalar.activation(out=y[:, eh * ND:(eh + 1) * ND],
                                     in_=ps[:], func=AF.Copy)
                nc.vector.bn_stats(out=stats[:, eh, :],
                                   in_=y[:, eh * ND:(eh + 1) * ND])
            mv, rstd = ln_scalars(stats)
            nc.vector.tensor_scalar(out=y[:], in0=y[:],
                                    scalar1=mv[:, 0:1], scalar2=rstd[:],
                                    op0=OP.subtract, op1=OP.mult)
            nc.gpsimd.tensor_mul(out=y[:], in0=y[:], in1=g2_rep[:])
            nc.vector.tensor_add(out=y[:], in0=y[:], in1=x_t[:])
            nc.sync.dma_start(out=out_r[lt], in_=y[:])
        h2T_pool.release()
        dramp.release()
        psum.release()
        statp.release()
        const.release()
        Fpool.release()

    nc.compile()
    return nc


def _bf(a):
    return np.ascontiguousarray(np.asarray(a, np.float32)).astype(_BF16)


def _prep_maps(inputs, L, D, n_cores):
    P = 128
    KT = D // P
    f32 = np.float32
    x = np.asarray(inputs["x"], f32)
    t = np.asarray(inputs["t"], f32)
    beta1 = float(np.asarray(inputs["beta1"], f32)[0])
    beta2 = float(np.asarray(inputs["beta2"], f32)[0])

    # SSM kernels -> mixed Toeplitz (transposed), host fp32
    af = np.diagonal(np.asarray(inputs["Af"], f32))
    ab = np.diagonal(np.asarray(inputs["Ab"], f32))
    l_ar = np.arange(L, dtype=f32)[:, None]
    kf = np.exp(l_ar * af[None, :]) @ (
        np.asarray(inputs["Bf"], f32)[:, 0] * np.asarray(inputs["Cf"], f32)[0]
    ) + np.asarray(inputs["Df"], f32)[0]
    kb = np.exp(l_ar * ab[None, :]) @ (
        np.asarray(inputs["Bb"], f32)[:, 0] * np.asarray(inputs["Cb"], f32)[0]
    ) + np.asarray(inputs["Db"], f32)[0]
    tms = np.arange(L)[None, :] - np.arange(L)[:, None]   # T_mixT[s,t] : t-s
    TmT = (np.where(tms >= 0, beta1 * kf[np.clip(tms, 0, None)], 0.0)
           + np.where(tms <= 0, beta2 * kb[np.clip(-tms, 0, None)], 0.0))
    TmT_bf = TmT.astype(f32).astype(_BF16)

    # timestep embedding -> noise scale (B, D)
    half = D // 2
    freqs = np.exp(np.arange(half, dtype=f32)
                   * (-math.log(10000.0) / (half - 1)))
    ang = t[:, None] * freqs[None, :]
    emb = np.concatenate([np.sin(ang), np.cos(ang)], axis=1).astype(f32)
    ns = (1.0 / (1.0 + np.exp(-emb))).astype(f32)         # (B, D)

    Wi_bf = _bf(inputs["Wi"])
    Wo_bf = _bf(inputs["Wo"])

    def conv_w(w):
        # (D_o, D_i, 3) -> [ot, p_i, it, j, o_local]
        w = np.asarray(w, f32).reshape(KT, P, KT, P, 3)
        return np.ascontiguousarray(
            w.transpose(0, 3, 2, 4, 1)).astype(_BF16)

    def col(v):
        return np.ascontiguousarray(np.asarray(v, f32).reshape(KT, P).T)

    shared = {
        "Wi": Wi_bf, "Wo": Wo_bf,
        "w1R": conv_w(inputs["w1"]), "w2R": conv_w(inputs["w2"]),
        "TmT": TmT_bf,
        "bc1c": col(inputs["bc1"]), "bc2c": col(inputs["bc2"]),
        "bi_row": _bf(inputs["bi"]).reshape(1, D),
        "bo_row": _bf(inputs["bo"]).reshape(1, D),
        "g1v": np.ascontiguousarray(np.asarray(inputs["g1"], f32)),
        "b1v": np.ascontiguousarray(np.asarray(inputs["b1"], f32)),
        "g2v": np.ascontiguousarray(np.asarray(inputs["g2"], f32)),
    }
    in_maps = []
    b2_fold = np.asarray(inputs["b2"], f32)[None, :]
    for b in range(n_cores):
        xb = np.ascontiguousarray(x[b])
        m = dict(shared)
        m["x_res"] = xb + b2_fold
        m["xT"] = np.ascontiguousarray(xb.T.astype(_BF16))
        m["nsc"] = np.ascontiguousarray(ns[b].reshape(KT, P).T)
        in_maps.append(m)
    return in_maps


def get_nc(L=_L, D=_D, n_cores=_B, debug_taps=False):
    key = (L, D, n_cores)
    if key not in _cache:
        _cache[key] = _build(L, D, n_cores)
    return _cache[key]


def kernel(**inputs):
    from concourse.bass_utils import run_bass_kernel_spmd

    L, D, B = _L, _D, _B
    nc = get_nc(L, D, B)
    in_maps = _prep_maps(inputs, L, D, B)
    res = run_bass_kernel_spmd(nc, in_maps, core_ids=list(range(B)))
    return np.stack([res.results[c]["out"] for c in range(B)]).astype(
        np.float32)


# revision 18
# speedup vs baseline: 4.5002x; 2.0464x over previous
"""Trainium2 Bass kernel for the DiffSSM block.

Data-parallel over batch B=8 across 8 NeuronCores (one batch element per
core). All heavy compute runs on the TensorEngine in bf16 with fp32 PSUM
accumulation; the tiny SSM kernel generation, timestep embedding, and
Toeplitz construction are host-side precompute.

v2 engine-balance redesign (vs the phase-serial baseline):
  - Bias adds folded into the matmul accumulation groups as K=1 matmuls
    (ones x bias_row), so LN stats run directly on PSUM.
  - LN normalize (x*rstd - mean*rstd) moved to ScalarE activation with
    per-partition scale/bias APs; only the g/b affine stays on VectorE.
  - Phase B (Toeplitz mix) evicts through ScalarE (Copy, scale=noise),
    phase E eviction stays on VectorE (scalar_tensor_tensor accumulate).
  - hln -> hlnT transpose done as 32 strip-wise xbar DMA transposes
    (per 512-row strip x 128-col block), overlapped with phase A.
  - Conv loops run ot-outer with double-buffered per-ot weight chunks
    (12 KB resident instead of 48 KB), double-buffered TmT chunks, and
    strip-buffered xT loads, so every phase's operands prefetch during
    the previous phase within the SBUF budget.

Device phases: A proj-in+LN1 -> B Toeplitz mix -> D conv1+Silu ->
E conv2 accumulate -> F proj-out+LN2+residual.
"""

import math

import numpy as np
import ml_dtypes

_BF16 = ml_dtypes.bfloat16

_L, _D, _B = 2048, 1024, 8

_cache = {}


def _build(L, D, n_cores):
    import concourse.bacc as bacc
    import concourse.bass as bass
    import concourse.tile as tile
    from concourse import mybir

    f32 = mybir.dt.float32
    bf16 = mybir.dt.bfloat16
    AF = mybir.ActivationFunctionType
    OP = mybir.AluOpType

    P = 128
    KT = D // P            # feature tiles
    LT = L // P            # sequence tiles
    ND = min(512, D)       # matmul free-dim chunk along features
    NF = min(512, L)       # matmul free-dim chunk along sequence
    EH = D // ND
    LC = L // NF
    ST = LT
    XSW = 256              # xT strip width
    SPL = XSW // P         # lt tiles per xT strip (2)
    TSW = 512              # transpose strip width (xbar free-dim mult 128)
    TPL = TSW // P         # lt tiles per transpose strip

    nc = bacc.Bacc("TRN2", target_bir_lowering=False, debug=False,
                   num_devices=n_cores)

    x_res = nc.dram_tensor("x_res", (L, D), f32, kind="ExternalInput").ap()
    xT = nc.dram_tensor("xT", (D, L), bf16, kind="ExternalInput").ap()
    Wi = nc.dram_tensor("Wi", (D, D), bf16, kind="ExternalInput").ap()
    w1R = nc.dram_tensor("w1R", (KT, P, KT, 3, P), bf16,
                         kind="ExternalInput").ap()
    w2R = nc.dram_tensor("w2R", (KT, P, KT, 3, P), bf16,
                         kind="ExternalInput").ap()
    Wo = nc.dram_tensor("Wo", (D, D), bf16, kind="ExternalInput").ap()
    TmT = nc.dram_tensor("TmT", (L, L), bf16, kind="ExternalInput").ap()
    nsc = nc.dram_tensor("nsc", (P, KT), f32, kind="ExternalInput").ap()
    bc1c = nc.dram_tensor("bc1c", (P, KT), f32, kind="ExternalInput").ap()
    bc2c = nc.dram_tensor("bc2c", (P, KT), f32, kind="ExternalInput").ap()
    bi_row = nc.dram_tensor("bi_row", (1, D), bf16, kind="ExternalInput").ap()
    bo_row = nc.dram_tensor("bo_row", (1, D), bf16, kind="ExternalInput").ap()
    vec_names = ["g1v", "b1v", "g2v"]
    vecs = {n: nc.dram_tensor(n, (D,), bf16, kind="ExternalInput").ap()
            for n in vec_names}
    out = nc.dram_tensor("out", (L, D), f32, kind="ExternalOutput").ap()

    with tile.TileContext(nc) as tc:
        # ---- pools (left stack, release order = reverse alloc) ----
        const = tc.alloc_tile_pool(name="const", bufs=1)
        statp = tc.alloc_tile_pool(name="stat", bufs=4)
        psum = tc.alloc_tile_pool(name="psum", bufs=8, space="PSUM")
        h2T_pool = tc.alloc_tile_pool(name="h2T", bufs=1)
        w2ch_pool = tc.alloc_tile_pool(name="w2ch", bufs=2)
        hlnT_pool = tc.alloc_tile_pool(name="hlnT", bufs=1)
        w1ch_pool = tc.alloc_tile_pool(name="w1ch", bufs=2)
        tb_pool = tc.alloc_tile_pool(name="tb", bufs=2)
        pa_pool = tc.alloc_tile_pool(name="pa", bufs=1)
        # right stack: hln (released end of B), then Fpool, co
        hln_pool = tc.alloc_tile_pool(name="hln", bufs=1, side="right")

        # ---- constants ----
        def rep_tile(name, pool=None):
            t = (pool or const).tile([P, D], bf16, tag=name,
                                     name=f"rep_{name}")
            ap = vecs[name]
            bcast = bass.AP(tensor=ap.tensor, offset=ap.offset,
                            ap=[[0, P]] + list(ap.ap))
            nc.gpsimd.dma_start(out=t[:], in_=bcast)
            return t

        g1_rep = rep_tile("g1v")
        b1_rep = rep_tile("b1v")
        ns_sb = const.tile([P, KT], f32)
        nc.sync.dma_start(out=ns_sb[:], in_=nsc)
        bc1_sb = const.tile([P, KT], f32)
        nc.sync.dma_start(out=bc1_sb[:], in_=bc1c)
        bc2_sb = const.tile([P, KT], f32)
        nc.sync.dma_start(out=bc2_sb[:], in_=bc2c)
        eps_sb = const.tile([P, 1], f32)
        nc.vector.memset(eps_sb[:], 1e-5)
        # preload the (large) sqrt activation table while initial DMAs run
        warm_sb = const.tile([P, 1], f32)
        nc.scalar.activation(out=warm_sb[:], in_=eps_sb[:], func=AF.Sqrt)
        ones_sb = const.tile([1, P], bf16)
        nc.vector.memset(ones_sb[:], 1.0)
        bi_sb = const.tile([1, D], bf16)
        nc.sync.dma_start(out=bi_sb[:], in_=bi_row)

        h2T_sb = h2T_pool.tile([P, KT, L], bf16)
        hlnT_sb = hlnT_pool.tile([P, KT, L], bf16)
        hln_sb = hln_pool.tile([P, LT, D], bf16)

        wi_sb = pa_pool.tile([P, KT, D], bf16, tag="wi")
        wi_r = Wi.rearrange("(kt p) d -> p kt d", p=P)
        xT_r = xT.rearrange("(kt p) l -> p kt l", p=P)
        Tm_r = TmT.rearrange("(st p) t -> p st t", p=P)

        def ln_scalars(stats_tile):
            """stats -> (mv, rstd) tiles."""
            mv = statp.tile([P, 2], f32, tag="mv", name="mv")
            nc.vector.bn_aggr(out=mv[:], in_=stats_tile[:])
            std = statp.tile([P, 1], f32, tag="std", name="std")
            nc.scalar.activation(out=std[:], in_=mv[:, 1:2], func=AF.Sqrt,
                                 bias=eps_sb[:], scale=1.0)
            rstd = statp.tile([P, 1], f32, tag="rstd", name="rstd")
            nc.vector.reciprocal(out=rstd[:], in_=std[:])
            return mv, rstd

        # ---- Phase A: proj-in + LN1 (stats on PSUM, norm on ScalarE) ----
        Tc_next = None
        xs = None
        for lt in range(LT):
            ls = lt // SPL
            if lt % SPL == 0:
                xs = pa_pool.tile([P, KT, XSW], bf16, tag="xs", name="xs",
                                  bufs=4)
                nc.gpsimd.dma_start(
                    out=xs[:],
                    in_=xT_r[:, :, ls * XSW:(ls + 1) * XSW])
            if lt == 0:
                nc.gpsimd.dma_start(out=wi_sb[:, :, 0:ND],
                                    in_=wi_r[:, :, 0:ND])
                nc.gpsimd.dma_start(out=wi_sb[:, :, ND:D],
                                    in_=wi_r[:, :, ND:D])
            if lt == 2:
                # prefetch first TmT chunk (needed only at phase B)
                Tc_next = tb_pool.tile([P, ST, NF], bf16, tag="Tc",
                                       name="Tc")
                nc.gpsimd.dma_start(out=Tc_next[:], in_=Tm_r[:, :, 0:NF])
            col = (lt % SPL) * P
            stats = statp.tile([P, EH, 6], f32, tag="stats", name="stats")
            nrm = statp.tile([P, D], bf16, tag="nrm", name="nrm", bufs=8)
            for eh in range(EH):
                ps = psum.tile([P, ND], f32, tag="ps", name="ps")
                nc.tensor.matmul(ps[:], lhsT=ones_sb[:],
                                 rhs=bi_sb[:, eh * ND:(eh + 1) * ND],
                                 start=True, stop=False)
                for kt in range(KT):
                    nc.tensor.matmul(ps[:],
                                     lhsT=xs[:, kt, col:col + P],
                                     rhs=wi_sb[:, kt, eh * ND:(eh + 1) * ND],
                                     start=False, stop=(kt == KT - 1))
                nc.scalar.activation(out=nrm[:, eh * ND:(eh + 1) * ND],
                                     in_=ps[:], func=AF.Copy)
                nc.vector.bn_stats(out=stats[:, eh, :],
                                   in_=nrm[:, eh * ND:(eh + 1) * ND])
            mv, rstd = ln_scalars(stats)
            nc.vector.tensor_scalar(out=nrm[:], in0=nrm[:],
                                    scalar1=mv[:, 0:1], scalar2=rstd[:],
                                    op0=OP.subtract, op1=OP.mult)
            nc.vector.tensor_mul(out=nrm[:], in0=nrm[:], in1=g1_rep[:])
            nc.vector.tensor_add(out=hln_sb[:, lt, :], in0=nrm[:],
                                 in1=b1_rep[:])
        # hln -> hlnT SBUF->SBUF xbar transposes, one per lt tile, issued
        # in REVERSE lt order: the in-order ACT sequencer then holds the
        # whole batch until A's last tile, so no transpose<->copy xbar-mode
        # alternation with A's loads; they run back-to-back at B's start
        # and only gate phase D.
        for lt in range(LT):
            nc.scalar.dma_start_transpose(
                out=hlnT_sb[:, :, lt * P:(lt + 1) * P],
                in_=hln_sb[:, lt, :])

        # ---- Phase B: SSM Toeplitz mix, evict via ScalarE * noise ----
        w1_next = None
        for tch in range(LC):
            Tc = Tc_next
            if tch + 1 < LC:
                Tc_next = tb_pool.tile([P, ST, NF], bf16, tag="Tc",
                                       name="Tc")
                nc.gpsimd.dma_start(
                    out=Tc_next[:],
                    in_=Tm_r[:, :, (tch + 1) * NF:(tch + 2) * NF])
            if tch == 0:
                # prefetch first conv1 weight chunk during B
                w1_next = w1ch_pool.tile([P, KT, 3, P], bf16, tag="w1c",
                                         name="w1c")
                nc.gpsimd.dma_start(out=w1_next[:], in_=w1R[0])
            for dt in range(KT):
                ps = psum.tile([P, NF], f32, tag="ps", name="ps")
                for st in range(ST):
                    nc.tensor.matmul(ps[:],
                                     lhsT=hln_sb[:, st, dt * P:(dt + 1) * P],
                                     rhs=Tc[:, st, :],
                                     start=(st == 0), stop=(st == ST - 1))
                nc.scalar.activation(
                    out=h2T_sb[:, dt, tch * NF:(tch + 1) * NF],
                    in_=ps[:], func=AF.Copy, scale=ns_sb[:, dt:dt + 1])
        pa_pool.release()
        tb_pool.release()
        hln_pool.release()

        # Fpool + co on the (now empty) right stack; loads overlap D/E.
        Fpool = tc.alloc_tile_pool(name="Fp", bufs=1, side="right")
        co_pool = tc.alloc_tile_pool(name="co", bufs=1, side="right")
        wo_sb = Fpool.tile([P, KT, D], bf16, tag="wo")
        wo_r = Wo.rearrange("(dt p) e -> dt p e", p=P)
        for dt in range(KT):
            nc.gpsimd.dma_start(out=wo_sb[:, dt, :], in_=wo_r[dt])
        bo_sb = Fpool.tile([1, D], bf16, tag="bo")
        nc.sync.dma_start(out=bo_sb[:], in_=bo_row)
        g2_rep = rep_tile("g2v", pool=Fpool)
        co_sb = co_pool.tile([P, KT, L], bf16)

        def conv_mms(ps, w_t, src_sb, lc):
            # kernel-3 conv as 3 shifted matmuls; j=1 (no shift) first so
            # start=True initializes the whole PSUM range; border columns
            # handled by narrowing the edge matmuls.
            first = True
            for it in range(KT):
                for j in (1, 0, 2):
                    o0 = 1 if (j == 0 and lc == 0) else 0
                    o1 = NF - 1 if (j == 2 and lc == LC - 1) else NF
                    base = lc * NF + j - 1
                    nc.tensor.matmul(
                        ps[:, o0:o1],
                        lhsT=w_t[:, it, j, :],
                        rhs=src_sb[:, it, base + o0:base + o1],
                        start=first,
                        stop=(it == KT - 1 and j == 2))
                    first = False

        # ---- Phase D: conv1 (+Silu), ot-outer with weight streaming ----
        w2_next = None
        for ot in range(KT):
            w1t = w1_next
            if ot + 1 < KT:
                w1_next = w1ch_pool.tile([P, KT, 3, P], bf16, tag="w1c",
                                         name="w1c")
                nc.gpsimd.dma_start(out=w1_next[:], in_=w1R[ot + 1])
            if ot == 0:
                w2_next = w2ch_pool.tile([P, KT, 3, P], bf16, tag="w2c",
                                         name="w2c")
                nc.gpsimd.dma_start(out=w2_next[:], in_=w2R[0])
            for lc in range(LC):
                ps = psum.tile([P, NF], f32, tag="ps", name="ps")
                conv_mms(ps, w1t, hlnT_sb, lc)
                nc.scalar.activation(
                    out=co_sb[:, ot, lc * NF:(lc + 1) * NF],
                    in_=ps[:], func=AF.Silu, bias=bc1_sb[:, ot:ot + 1],
                    scale=1.0)
        w1ch_pool.release()
        hlnT_pool.release()

        # ---- Phase E: conv2, accumulate into h2T ----
        for ot in range(KT):
            w2t = w2_next
            if ot + 1 < KT:
                w2_next = w2ch_pool.tile([P, KT, 3, P], bf16, tag="w2c",
                                         name="w2c")
                nc.gpsimd.dma_start(out=w2_next[:], in_=w2R[ot + 1])
            for lc in range(LC):
                ps = psum.tile([P, NF], f32, tag="ps", name="ps")
                conv_mms(ps, w2t, co_sb, lc)
                nc.vector.scalar_tensor_tensor(
                    out=h2T_sb[:, ot, lc * NF:(lc + 1) * NF],
                    in0=ps[:], scalar=bc2_sb[:, ot:ot + 1],
                    in1=h2T_sb[:, ot, lc * NF:(lc + 1) * NF],
                    op0=OP.add, op1=OP.add)
        w2ch_pool.release()
        co_pool.release()

        # ---- Phase F: proj-out + LN2 + residual ----
        x_r = x_res.rearrange("(t p) d -> t p d", p=P)
        out_r = out.rearrange("(t p) d -> t p d", p=P)
        for lt in range(LT):
            x_t = Fpool.tile([P, D], f32, tag="x_t", name="x_t", bufs=2)
            nc.sync.dma_start(out=x_t[:], in_=x_r[lt])
            stats = statp.tile([P, EH, 6], f32, tag="stats", name="stats")
            y = Fpool.tile([P, D], bf16, tag="y", name="y", bufs=4)
            for eh in range(EH):
                ps = psum.tile([P, ND], f32, tag="ps", name="ps")
                nc.tensor.matmul(ps[:], lhsT=ones_sb[:],
                                 rhs=bo_sb[:, eh * ND:(eh + 1) * ND],
                                 start=True, stop=False)
                for dt in range(KT):
                    nc.tensor.matmul(ps[:],
                                     lhsT=h2T_sb[:, dt, lt * P:(lt + 1) * P],
                                     rhs=wo_sb[:, dt, eh * ND:(eh + 1) * ND],
                                     start=False, stop=(dt == KT - 1))
                nc.scalar.activation(out=y[:, eh * ND:(eh + 1) * ND],
                                     in_=ps[:], func=AF.Copy)
                nc.vector.bn_stats(out=stats[:, eh, :],
                                   in_=y[:, eh * ND:(eh + 1) * ND])
            mv, rstd = ln_scalars(stats)
            nc.vector.tensor_scalar(out=y[:], in0=y[:],
                                    scalar1=mv[:, 0:1], scalar2=rstd[:],
                                    op0=OP.subtract, op1=OP.mult)
            nc.vector.tensor_mul(out=y[:], in0=y[:], in1=g2_rep[:])
            nc.vector.tensor_add(out=x_t[:], in0=y[:], in1=x_t[:])
            nc.sync.dma_start(out=out_r[lt], in_=x_t[:])
        h2T_pool.release()
        psum.release()
        statp.release()
        const.release()
        Fpool.release()

    nc.compile()
    return nc


def _bf(a):
    return np.ascontiguousarray(np.asarray(a, np.float32)).astype(_BF16)


def _prep_maps(inputs, L, D, n_cores):
    P = 128
    KT = D // P
    f32 = np.float32
    x = np.asarray(inputs["x"], f32)
    t = np.asarray(inputs["t"], f32)
    beta1 = float(np.asarray(inputs["beta1"], f32)[0])
    beta2 = float(np.asarray(inputs["beta2"], f32)[0])

    # SSM kernels -> mixed Toeplitz (transposed), host fp32
    af = np.diagonal(np.asarray(inputs["Af"], f32))
    ab = np.diagonal(np.asarray(inputs["Ab"], f32))
    l_ar = np.arange(L, dtype=f32)[:, None]
    kf = np.exp(l_ar * af[None, :]) @ (
        np.asarray(inputs["Bf"], f32)[:, 0] * np.asarray(inputs["Cf"], f32)[0]
    ) + np.asarray(inputs["Df"], f32)[0]
    kb = np.exp(l_ar * ab[None, :]) @ (
        np.asarray(inputs["Bb"], f32)[:, 0] * np.asarray(inputs["Cb"], f32)[0]
    ) + np.asarray(inputs["Db"], f32)[0]
    tms = np.arange(L)[None, :] - np.arange(L)[:, None]   # T_mixT[s,t] : t-s
    TmT = (np.where(tms >= 0, beta1 * kf[np.clip(tms, 0, None)], 0.0)
           + np.where(tms <= 0, beta2 * kb[np.clip(-tms, 0, None)], 0.0))
    TmT_bf = TmT.astype(f32).astype(_BF16)

    # timestep embedding -> noise scale (B, D)
    half = D // 2
    freqs = np.exp(np.arange(half, dtype=f32)
                   * (-math.log(10000.0) / (half - 1)))
    ang = t[:, None] * freqs[None, :]
    emb = np.concatenate([np.sin(ang), np.cos(ang)], axis=1).astype(f32)
    ns = (1.0 / (1.0 + np.exp(-emb))).astype(f32)         # (B, D)

    Wi_bf = _bf(inputs["Wi"])
    Wo_bf = _bf(inputs["Wo"])

    def conv_w(w):
        # (D_o, D_i, 3) -> [ot, p_i, it, j, o_local]
        w = np.asarray(w, f32).reshape(KT, P, KT, P, 3)
        return np.ascontiguousarray(
            w.transpose(0, 3, 2, 4, 1)).astype(_BF16)

    def col(v):
        return np.ascontiguousarray(np.asarray(v, f32).reshape(KT, P).T)

    shared = {
        "Wi": Wi_bf, "Wo": Wo_bf,
        "w1R": conv_w(inputs["w1"]), "w2R": conv_w(inputs["w2"]),
        "TmT": TmT_bf,
        "bc1c": col(inputs["bc1"]), "bc2c": col(inputs["bc2"]),
        "bi_row": _bf(inputs["bi"]).reshape(1, D),
        "bo_row": _bf(inputs["bo"]).reshape(1, D),
        "g1v": _bf(inputs["g1"]),
        "b1v": _bf(inputs["b1"]),
        "g2v": _bf(inputs["g2"]),
    }
    in_maps = []
    b2_fold = np.asarray(inputs["b2"], f32)[None, :]
    for b in range(n_cores):
        xb = np.ascontiguousarray(x[b])
        m = dict(shared)
        m["x_res"] = xb + b2_fold
        m["xT"] = np.ascontiguousarray(xb.T.astype(_BF16))
        m["nsc"] = np.ascontiguousarray(ns[b].reshape(KT, P).T)
        in_maps.append(m)
    return in_maps


def get_nc(L=_L, D=_D, n_cores=_B, debug_taps=False):
    key = (L, D, n_cores)
    if key not in _cache:
        _cache[key] = _build(L, D, n_cores)
    return _cache[key]


def kernel(**inputs):
    from concourse.bass_utils import run_bass_kernel_spmd

    L, D, B = _L, _D, _B
    nc = get_nc(L, D, B)
    in_maps = _prep_maps(inputs, L, D, B)
    res = run_bass_kernel_spmd(nc, in_maps, core_ids=list(range(B)))
    return np.stack([res.results[c]["out"] for c in range(B)]).astype(
        np.float32)


# revision 28
# speedup vs baseline: 5.3289x; 1.1841x over previous
"""Trainium2 Bass kernel for the DiffSSM block.

Data-parallel over batch B=8 across 8 NeuronCores (one batch element per
core). All heavy compute runs on the TensorEngine in bf16 with fp32 PSUM
accumulation; the tiny SSM kernel generation, timestep embedding, and
Toeplitz construction are host-side precompute.

v2 engine-balance redesign (vs the phase-serial baseline):
  - Bias adds folded into the matmul accumulation groups as K=1 matmuls
    (ones x bias_row), so LN stats run directly on PSUM.
  - LN normalize (x*rstd - mean*rstd) moved to ScalarE activation with
    per-partition scale/bias APs; only the g/b affine stays on VectorE.
  - Phase B (Toeplitz mix) evicts through ScalarE (Copy, scale=noise),
    phase E eviction stays on VectorE (scalar_tensor_tensor accumulate).
  - hln -> hlnT transpose done as 32 strip-wise xbar DMA transposes
    (per 512-row strip x 128-col block), overlapped with phase A.
  - Conv loops run ot-outer with double-buffered per-ot weight chunks
    (12 KB resident instead of 48 KB), double-buffered TmT chunks, and
    strip-buffered xT loads, so every phase's operands prefetch during
    the previous phase within the SBUF budget.

Device phases: A proj-in+LN1 -> B Toeplitz mix -> D conv1+Silu ->
E conv2 accumulate -> F proj-out+LN2+residual.
"""

import math

import numpy as np
import ml_dtypes

_BF16 = ml_dtypes.bfloat16

_L, _D, _B = 2048, 1024, 8

_cache = {}


def _build(L, D, n_cores):
    import concourse.bacc as bacc
    import concourse.bass as bass
    import concourse.tile as tile
    from concourse import mybir

    f32 = mybir.dt.float32
    bf16 = mybir.dt.bfloat16
    AF = mybir.ActivationFunctionType
    OP = mybir.AluOpType

    P = 128
    KT = D // P            # feature tiles
    LT = L // P            # sequence tiles
    ND = min(512, D)       # matmul free-dim chunk along features
    NF = min(512, L)       # matmul free-dim chunk along sequence
    EH = D // ND
    LC = L // NF
    ST = LT
    XSW = 256              # xT strip width
    SPL = XSW // P         # lt tiles per xT strip (2)
    TSW = 512              # transpose strip width (xbar free-dim mult 128)
    TPL = TSW // P         # lt tiles per transpose strip

    nc = bacc.Bacc("TRN2", target_bir_lowering=False, debug=False,
                   num_devices=n_cores)

    x_res = nc.dram_tensor("x_res", (L, D), f32, kind="ExternalInput").ap()
    xT = nc.dram_tensor("xT", (D, L), bf16, kind="ExternalInput").ap()
    Wi = nc.dram_tensor("Wi", (D, D), bf16, kind="ExternalInput").ap()
    w1R = nc.dram_tensor("w1R", (KT, P, KT, 3, P), bf16,
                         kind="ExternalInput").ap()
    w2R = nc.dram_tensor("w2R", (KT, P, KT, 3, P), bf16,
                         kind="ExternalInput").ap()
    Wo = nc.dram_tensor("Wo", (D, D), bf16, kind="ExternalInput").ap()
    NS = 72                # SSM states (64 modes + Df/Db const + pad)
    QC = 512               # SSD chunk length
    NCH = L // QC          # chunks
    SPC = QC // 128        # 128-tiles per chunk
    TmD = nc.dram_tensor("TmD", (NCH, 128, SPC, QC), bf16,
                         kind="ExternalInput").ap()
    AfP = nc.dram_tensor("AfP", (128, SPC, NS), bf16,
                         kind="ExternalInput").ap()
    AbP = nc.dram_tensor("AbP", (128, SPC, NS), bf16,
                         kind="ExternalInput").ap()
    CfO = nc.dram_tensor("CfO", (NS, QC), bf16, kind="ExternalInput").ap()
    CbO = nc.dram_tensor("CbO", (NS, QC), bf16, kind="ExternalInput").ap()
    lamf = nc.dram_tensor("lamf", (NS, 1), f32, kind="ExternalInput").ap()
    lamb = nc.dram_tensor("lamb", (NS, 1), f32, kind="ExternalInput").ap()
    nsc = nc.dram_tensor("nsc", (P, KT), f32, kind="ExternalInput").ap()
    bc1c = nc.dram_tensor("bc1c", (P, KT), f32, kind="ExternalInput").ap()
    bc2c = nc.dram_tensor("bc2c", (P, KT), f32, kind="ExternalInput").ap()
    bi_row = nc.dram_tensor("bi_row", (1, D), bf16, kind="ExternalInput").ap()
    bo_row = nc.dram_tensor("bo_row", (1, D), bf16, kind="ExternalInput").ap()
    vec_names = ["g1v", "b1v", "g2v"]
    vecs = {n: nc.dram_tensor(n, (D,), bf16, kind="ExternalInput").ap()
            for n in vec_names}
    out = nc.dram_tensor("out", (L, D), f32, kind="ExternalOutput").ap()

    with tile.TileContext(nc) as tc:
        # ---- pools (left stack, release order = reverse alloc) ----
        const = tc.alloc_tile_pool(name="const", bufs=1)
        statp = tc.alloc_tile_pool(name="stat", bufs=4)
        psum = tc.alloc_tile_pool(name="psum", bufs=8, space="PSUM")
        h2T_pool = tc.alloc_tile_pool(name="h2T", bufs=1)
        w2ch_pool = tc.alloc_tile_pool(name="w2ch", bufs=2)
        hlnT_pool = tc.alloc_tile_pool(name="hlnT", bufs=1)
        w1ch_pool = tc.alloc_tile_pool(name="w1ch", bufs=2)
        tb_pool = tc.alloc_tile_pool(name="tb", bufs=2)
        pa_pool = tc.alloc_tile_pool(name="pa", bufs=1)
        # right stack: hln (released end of B), then Fpool, co
        hln_pool = tc.alloc_tile_pool(name="hln", bufs=1, side="right")

        # ---- constants ----
        def rep_tile(name, pool=None):
            t = (pool or const).tile([P, D], bf16, tag=name,
                                     name=f"rep_{name}")
            ap = vecs[name]
            bcast = bass.AP(tensor=ap.tensor, offset=ap.offset,
                            ap=[[0, P]] + list(ap.ap))
            nc.gpsimd.dma_start(out=t[:], in_=bcast)
            return t

        bi_sb = const.tile([1, D], bf16)
        nc.sync.dma_start(out=bi_sb[:], in_=bi_row)
        eps_sb = const.tile([P, 1], f32)
        nc.vector.memset(eps_sb[:], 1e-5)
        ones_sb = const.tile([1, P], bf16)
        nc.vector.memset(ones_sb[:], 1.0)
        # preload the (large) sqrt activation table while initial DMAs run
        warm_sb = const.tile([P, 1], f32)
        nc.scalar.activation(out=warm_sb[:], in_=eps_sb[:], func=AF.Sqrt)
        g1_rep = rep_tile("g1v")
        b1_rep = rep_tile("b1v")
        ns_sb = const.tile([P, KT], f32)
        nc.sync.dma_start(out=ns_sb[:], in_=nsc)
        bc1_sb = const.tile([P, KT], f32)
        nc.sync.dma_start(out=bc1_sb[:], in_=bc1c)
        bc2_sb = const.tile([P, KT], f32)
        nc.sync.dma_start(out=bc2_sb[:], in_=bc2c)
        AfP_sb = const.tile([P, SPC, NS], bf16)
        nc.sync.dma_start(out=AfP_sb[:], in_=AfP)
        AbP_sb = const.tile([P, SPC, NS], bf16)
        nc.sync.dma_start(out=AbP_sb[:], in_=AbP)
        CfO_sb = const.tile([NS, QC], bf16)
        nc.sync.dma_start(out=CfO_sb[:], in_=CfO)
        CbO_sb = const.tile([NS, QC], bf16)
        nc.sync.dma_start(out=CbO_sb[:], in_=CbO)
        lamf_sb = const.tile([NS, 1], f32)
        nc.sync.dma_start(out=lamf_sb[:], in_=lamf)
        lamb_sb = const.tile([NS, 1], f32)
        nc.sync.dma_start(out=lamb_sb[:], in_=lamb)

        h2T_sb = h2T_pool.tile([P, KT, L], bf16)
        hlnT_sb = hlnT_pool.tile([P, KT, L], bf16)
        hln_sb = hln_pool.tile([P, LT, D], bf16)

        wi_sb = pa_pool.tile([P, KT, D], bf16, tag="wi")
        wi_r = Wi.rearrange("(kt p) d -> p kt d", p=P)
        xT_r = xT.rearrange("(kt p) l -> p kt l", p=P)

        def ln_scalars(stats_tile):
            """stats -> (mv, rstd) tiles."""
            mv = statp.tile([P, 2], f32, tag="mv", name="mv")
            nc.vector.bn_aggr(out=mv[:], in_=stats_tile[:])
            std = statp.tile([P, 1], f32, tag="std", name="std")
            nc.scalar.activation(out=std[:], in_=mv[:, 1:2], func=AF.Sqrt,
                                 bias=eps_sb[:], scale=1.0)
            rstd = statp.tile([P, 1], f32, tag="rstd", name="rstd")
            nc.vector.reciprocal(out=rstd[:], in_=std[:])
            return mv, rstd

        # ---- Phase A: proj-in + LN1 (stats on PSUM, norm on ScalarE) ----
        Tc_next = None
        xs = None
        for lt in range(LT):
            ls = lt // SPL
            if lt % SPL == 0:
                xs = pa_pool.tile([P, KT, XSW], bf16, tag="xs", name="xs",
                                  bufs=4)
                nc.gpsimd.dma_start(
                    out=xs[:],
                    in_=xT_r[:, :, ls * XSW:(ls + 1) * XSW])
            if lt == 0:
                nc.gpsimd.dma_start(out=wi_sb[:, :, 0:ND],
                                    in_=wi_r[:, :, 0:ND])
                nc.gpsimd.dma_start(out=wi_sb[:, :, ND:D],
                                    in_=wi_r[:, :, ND:D])
            if lt == 2:
                # prefetch first Toeplitz diag block (needed at phase B)
                Tc_next = tb_pool.tile([P, SPC, QC], bf16, tag="Tc",
                                       name="Tc")
                nc.gpsimd.dma_start(out=Tc_next[:], in_=TmD[0])
            col = (lt % SPL) * P
            stats = statp.tile([P, EH, 6], f32, tag="stats", name="stats")
            nrm = statp.tile([P, D], bf16, tag="nrm", name="nrm", bufs=8)
            for eh in range(EH):
                ps = psum.tile([P, ND], f32, tag="ps", name="ps")
                nc.tensor.matmul(ps[:], lhsT=ones_sb[:],
                                 rhs=bi_sb[:, eh * ND:(eh + 1) * ND],
                                 start=True, stop=False)
                for kt in range(KT):
                    nc.tensor.matmul(ps[:],
                                     lhsT=xs[:, kt, col:col + P],
                                     rhs=wi_sb[:, kt, eh * ND:(eh + 1) * ND],
                                     start=False, stop=(kt == KT - 1))
                nc.scalar.activation(out=nrm[:, eh * ND:(eh + 1) * ND],
                                     in_=ps[:], func=AF.Copy)
                nc.vector.bn_stats(out=stats[:, eh, :],
                                   in_=nrm[:, eh * ND:(eh + 1) * ND])
            mv, rstd = ln_scalars(stats)
            nc.vector.tensor_scalar(out=nrm[:], in0=nrm[:],
                                    scalar1=mv[:, 0:1], scalar2=rstd[:],
                                    op0=OP.subtract, op1=OP.mult)
            nc.vector.tensor_mul(out=nrm[:], in0=nrm[:], in1=g1_rep[:])
            nc.vector.tensor_add(out=hln_sb[:, lt, :], in0=nrm[:],
                                 in1=b1_rep[:])
        # hln -> hlnT SBUF->SBUF xbar transposes, one per lt tile. The
        # xbar-mode switch serializes against ALL in-flight DMA copies, so
        # a transpose scheduled mid-A stalls the ACT sequencer for ~10us.
        # Guard: write one byte into every transpose's output block, with
        # the guard reading A's last hln tile -- every transpose then
        # WAW-depends on A being fully done and the batch runs back-to-back
        # at B's start (gating only phase D).
        hlnT_r = hlnT_sb[:].rearrange("p kt (lt c) -> p kt lt c", c=P)
        nc.scalar.activation(out=hlnT_r[:, 0, :, 0:1],
                             in_=hln_sb[:, LT - 1, 0:LT], func=AF.Copy)
        for lt in range(LT):
            nc.scalar.dma_start_transpose(
                out=hlnT_sb[:, :, lt * P:(lt + 1) * P],
                in_=hln_sb[:, lt, :])

        # ---- Phase B: SSD chunked SSM mix ----
        # cross-chunk states: Zf[c] = sum_{cs<c} Lam^(Q(c-cs-1)) Pf[cs],
        # Gb[c] = sum_{cs>c} Lam^(Q(cs-c-1)) Pb[cs]; recurrences fused into
        # the PSUM evictions (scalar_tensor_tensor).
        def state_proj(proj_sb, c):
            pss = []
            for eh in range(EH):
                ps = psum.tile([NS, ND], f32, tag="ps", name="psP")
                for st in range(SPC):
                    nc.tensor.matmul(
                        ps[:], lhsT=proj_sb[:, st, :],
                        rhs=hln_sb[:, c * SPC + st, eh * ND:(eh + 1) * ND],
                        start=(st == 0), stop=(st == SPC - 1))
                pss.append(ps)
            return pss

        def state_tile(name):
            return tb_pool.tile([NS, D], bf16, tag=name, name=name, bufs=1)

        Zf = {}
        for c in range(NCH - 1):
            pss = state_proj(AfP_sb, c)
            Zf[c + 1] = state_tile(f"Zf{c + 1}")
            for eh in range(EH):
                sl = slice(eh * ND, (eh + 1) * ND)
                if c == 0:
                    nc.vector.tensor_copy(out=Zf[1][:, sl],
                                          in_=pss[eh][:])
                else:
                    nc.vector.scalar_tensor_tensor(
                        out=Zf[c + 1][:, sl], in0=Zf[c][:, sl],
                        scalar=lamf_sb[:, 0:1], in1=pss[eh][:],
                        op0=OP.mult, op1=OP.add)
        Gb = {}
        for c in range(NCH - 1, 0, -1):
            pss = state_proj(AbP_sb, c)
            Gb[c - 1] = state_tile(f"Gb{c - 1}")
            for eh in range(EH):
                sl = slice(eh * ND, (eh + 1) * ND)
                if c == NCH - 1:
                    nc.vector.tensor_copy(out=Gb[c - 1][:, sl],
                                          in_=pss[eh][:])
                else:
                    nc.vector.scalar_tensor_tensor(
                        out=Gb[c - 1][:, sl], in0=Gb[c][:, sl],
                        scalar=lamb_sb[:, 0:1], in1=pss[eh][:],
                        op0=OP.mult, op1=OP.add)

        w1_next = None
        for tch in range(NCH):
            Tc = Tc_next
            if tch + 1 < NCH:
                Tc_next = tb_pool.tile([P, SPC, QC], bf16, tag="Tc",
                                       name="Tc")
                nc.gpsimd.dma_start(out=Tc_next[:], in_=TmD[tch + 1])
            if tch == 0:
                # prefetch first conv1 weight chunk during B
                w1_next = w1ch_pool.tile([P, KT, 3, P], bf16, tag="w1c",
                                         name="w1c")
                nc.gpsimd.dma_start(out=w1_next[:], in_=w1R[0])
            for dt in range(KT):
                ps = psum.tile([P, QC], f32, tag="ps", name="ps")
                n_mm = SPC + (tch > 0) + (tch < NCH - 1)
                k = 0
                for st in range(SPC):
                    k += 1
                    nc.tensor.matmul(
                        ps[:],
                        lhsT=hln_sb[:, tch * SPC + st, dt * P:(dt + 1) * P],
                        rhs=Tc[:, st, :],
                        start=(st == 0), stop=(k == n_mm))
                if tch > 0:
                    k += 1
                    nc.tensor.matmul(ps[:],
                                     lhsT=Zf[tch][:, dt * P:(dt + 1) * P],
                                     rhs=CfO_sb[:], start=False,
                                     stop=(k == n_mm))
                if tch < NCH - 1:
                    k += 1
                    nc.tensor.matmul(ps[:],
                                     lhsT=Gb[tch][:, dt * P:(dt + 1) * P],
                                     rhs=CbO_sb[:], start=False,
                                     stop=(k == n_mm))
                nc.vector.tensor_scalar_mul(
                    out=h2T_sb[:, dt, tch * QC:(tch + 1) * QC],
                    in0=ps[:], scalar1=ns_sb[:, dt:dt + 1])
        pa_pool.release()
        tb_pool.release()
        hln_pool.release()

        # Fpool + co on the (now empty) right stack; loads overlap D/E.
        Fpool = tc.alloc_tile_pool(name="Fp", bufs=1, side="right")
        co_pool = tc.alloc_tile_pool(name="co", bufs=1, side="right")
        wo_sb = Fpool.tile([P, KT, D], bf16, tag="wo")
        wo_r = Wo.rearrange("(dt p) e -> dt p e", p=P)
        for dt in range(KT):
            nc.gpsimd.dma_start(out=wo_sb[:, dt, :], in_=wo_r[dt])
        bo_sb = Fpool.tile([1, D], bf16, tag="bo")
        nc.sync.dma_start(out=bo_sb[:], in_=bo_row)
        g2_rep = rep_tile("g2v", pool=Fpool)
        co_sb = co_pool.tile([P, KT, L], bf16)

        def conv_mms(ps, w_t, src_sb, lc):
            # kernel-3 conv as 3 shifted matmuls; j=1 (no shift) first so
            # start=True initializes the whole PSUM range; border columns
            # handled by narrowing the edge matmuls.
            first = True
            for it in range(KT):
                for j in (1, 0, 2):
                    o0 = 1 if (j == 0 and lc == 0) else 0
                    o1 = NF - 1 if (j == 2 and lc == LC - 1) else NF
                    base = lc * NF + j - 1
                    nc.tensor.matmul(
                        ps[:, o0:o1],
                        lhsT=w_t[:, it, j, :],
                        rhs=src_sb[:, it, base + o0:base + o1],
                        start=first,
                        stop=(it == KT - 1 and j == 2))
                    first = False

        # ---- Phase D: conv1 (+Silu), ot-outer with weight streaming ----
        w2_next = None
        for ot in range(KT):
            w1t = w1_next
            if ot + 1 < KT:
                w1_next = w1ch_pool.tile([P, KT, 3, P], bf16, tag="w1c",
                                         name="w1c")
                nc.gpsimd.dma_start(out=w1_next[:], in_=w1R[ot + 1])
            if ot == 0:
                w2_next = w2ch_pool.tile([P, KT, 3, P], bf16, tag="w2c",
                                         name="w2c")
                nc.gpsimd.dma_start(out=w2_next[:], in_=w2R[0])
            for lc in range(LC):
                ps = psum.tile([P, NF], f32, tag="ps", name="ps")
                conv_mms(ps, w1t, hlnT_sb, lc)
                nc.scalar.activation(
                    out=co_sb[:, ot, lc * NF:(lc + 1) * NF],
                    in_=ps[:], func=AF.Silu, bias=bc1_sb[:, ot:ot + 1],
                    scale=1.0)
        w1ch_pool.release()
        hlnT_pool.release()

        # ---- Phase E: conv2, accumulate into h2T ----
        for ot in range(KT):
            w2t = w2_next
            if ot + 1 < KT:
                w2_next = w2ch_pool.tile([P, KT, 3, P], bf16, tag="w2c",
                                         name="w2c")
                nc.gpsimd.dma_start(out=w2_next[:], in_=w2R[ot + 1])
            for lc in range(LC):
                ps = psum.tile([P, NF], f32, tag="ps", name="ps")
                conv_mms(ps, w2t, co_sb, lc)
                nc.vector.scalar_tensor_tensor(
                    out=h2T_sb[:, ot, lc * NF:(lc + 1) * NF],
                    in0=ps[:], scalar=bc2_sb[:, ot:ot + 1],
                    in1=h2T_sb[:, ot, lc * NF:(lc + 1) * NF],
                    op0=OP.add, op1=OP.add)
        w2ch_pool.release()
        co_pool.release()

        # ---- Phase F: proj-out + LN2 + residual ----
        x_r = x_res.rearrange("(t p) d -> t p d", p=P)
        out_r = out.rearrange("(t p) d -> t p d", p=P)
        for lt in range(LT):
            x_t = Fpool.tile([P, D], f32, tag="x_t", name="x_t", bufs=2)
            nc.sync.dma_start(out=x_t[:], in_=x_r[lt])
            stats = statp.tile([P, EH, 6], f32, tag="stats", name="stats")
            y = Fpool.tile([P, D], bf16, tag="y", name="y", bufs=4)
            for eh in range(EH):
                ps = psum.tile([P, ND], f32, tag="ps", name="ps")
                nc.tensor.matmul(ps[:], lhsT=ones_sb[:],
                                 rhs=bo_sb[:, eh * ND:(eh + 1) * ND],
                                 start=True, stop=False)
                for dt in range(KT):
                    nc.tensor.matmul(ps[:],
                                     lhsT=h2T_sb[:, dt, lt * P:(lt + 1) * P],
                                     rhs=wo_sb[:, dt, eh * ND:(eh + 1) * ND],
                                     start=False, stop=(dt == KT - 1))
                nc.scalar.activation(out=y[:, eh * ND:(eh + 1) * ND],
                                     in_=ps[:], func=AF.Copy)
                nc.vector.bn_stats(out=stats[:, eh, :],
                                   in_=y[:, eh * ND:(eh + 1) * ND])
            mv, rstd = ln_scalars(stats)
            nc.vector.tensor_scalar(out=y[:], in0=y[:],
                                    scalar1=mv[:, 0:1], scalar2=rstd[:],
                                    op0=OP.subtract, op1=OP.mult)
            nc.vector.tensor_mul(out=y[:], in0=y[:], in1=g2_rep[:])
            nc.vector.tensor_add(out=x_t[:], in0=y[:], in1=x_t[:])
            nc.sync.dma_start(out=out_r[lt], in_=x_t[:])
        h2T_pool.release()
        psum.release()
        statp.release()
        const.release()
        Fpool.release()

    nc.compile()
    return nc


def _bf(a):
    return np.ascontiguousarray(np.asarray(a, np.float32)).astype(_BF16)


def _prep_maps(inputs, L, D, n_cores):
    P = 128
    KT = D // P
    f32 = np.float32
    x = np.asarray(inputs["x"], f32)
    t = np.asarray(inputs["t"], f32)
    beta1 = float(np.asarray(inputs["beta1"], f32)[0])
    beta2 = float(np.asarray(inputs["beta2"], f32)[0])

    # SSM kernels -> mixed Toeplitz (transposed), host fp32
    af = np.diagonal(np.asarray(inputs["Af"], f32))
    ab = np.diagonal(np.asarray(inputs["Ab"], f32))
    l_ar = np.arange(L, dtype=f32)[:, None]
    kf = np.exp(l_ar * af[None, :]) @ (
        np.asarray(inputs["Bf"], f32)[:, 0] * np.asarray(inputs["Cf"], f32)[0]
    ) + np.asarray(inputs["Df"], f32)[0]
    kb = np.exp(l_ar * ab[None, :]) @ (
        np.asarray(inputs["Bb"], f32)[:, 0] * np.asarray(inputs["Cb"], f32)[0]
    ) + np.asarray(inputs["Db"], f32)[0]
    # within-chunk mixed Toeplitz diagonal blocks (exact)
    QC, NS = 512, 72
    NCH = L // QC
    tms = np.arange(QC)[None, :] - np.arange(QC)[:, None]  # [s_loc, t_loc]
    TmQ = (np.where(tms >= 0, beta1 * kf[np.clip(tms, 0, None)], 0.0)
           + np.where(tms <= 0, beta2 * kb[np.clip(-tms, 0, None)], 0.0))
    TmD = np.broadcast_to(
        TmQ.reshape(1, QC // 128, 128, QC).transpose(0, 2, 1, 3),
        (NCH, 128, QC // 128, QC))
    TmD = np.ascontiguousarray(TmD).astype(f32).astype(_BF16)
    # cross-chunk rank-NS state matrices (64 modes + const Df/Db state)
    wf = (np.asarray(inputs["Bf"], f32)[:, 0]
          * np.asarray(inputs["Cf"], f32)[0])
    wb = (np.asarray(inputs["Bb"], f32)[:, 0]
          * np.asarray(inputs["Cb"], f32)[0])
    Df = float(np.asarray(inputs["Df"], f32)[0])
    Db = float(np.asarray(inputs["Db"], f32)[0])
    s_loc = np.arange(QC, dtype=f32)
    AfP = np.zeros((QC, NS), f32)
    AfP[:, :64] = np.exp((QC - 1 - s_loc)[:, None] * af[None, :])
    AfP[:, 64] = 1.0
    AbP = np.zeros((QC, NS), f32)
    AbP[:, :64] = np.exp((s_loc + 1)[:, None] * ab[None, :])
    AbP[:, 64] = 1.0
    t_loc = np.arange(QC, dtype=f32)
    CfO = np.zeros((NS, QC), f32)
    CfO[:64] = beta1 * wf[:, None] * np.exp(af[:, None] * (t_loc + 1)[None])
    CfO[64] = beta1 * Df
    CbO = np.zeros((NS, QC), f32)
    CbO[:64] = beta2 * wb[:, None] * np.exp(
        ab[:, None] * (QC - 1 - t_loc)[None])
    CbO[64] = beta2 * Db
    lamf = np.zeros((NS, 1), f32)
    lamf[:64, 0] = np.exp(af * QC)
    lamf[64, 0] = 1.0
    lamb = np.zeros((NS, 1), f32)
    lamb[:64, 0] = np.exp(ab * QC)
    lamb[64, 0] = 1.0
    AfP_d = np.ascontiguousarray(
        AfP.reshape(QC // 128, 128, NS).transpose(1, 0, 2)).astype(_BF16)
    AbP_d = np.ascontiguousarray(
        AbP.reshape(QC // 128, 128, NS).transpose(1, 0, 2)).astype(_BF16)

    # timestep embedding -> noise scale (B, D)
    half = D // 2
    freqs = np.exp(np.arange(half, dtype=f32)
                   * (-math.log(10000.0) / (half - 1)))
    ang = t[:, None] * freqs[None, :]
    emb = np.concatenate([np.sin(ang), np.cos(ang)], axis=1).astype(f32)
    ns = (1.0 / (1.0 + np.exp(-emb))).astype(f32)         # (B, D)

    Wi_bf = _bf(inputs["Wi"])
    Wo_bf = _bf(inputs["Wo"])

    def conv_w(w):
        # (D_o, D_i, 3) -> [ot, p_i, it, j, o_local]
        w = np.asarray(w, f32).reshape(KT, P, KT, P, 3)
        return np.ascontiguousarray(
            w.transpose(0, 3, 2, 4, 1)).astype(_BF16)

    def col(v):
        return np.ascontiguousarray(np.asarray(v, f32).reshape(KT, P).T)

    shared = {
        "Wi": Wi_bf, "Wo": Wo_bf,
        "w1R": conv_w(inputs["w1"]), "w2R": conv_w(inputs["w2"]),
        "TmD": TmD, "AfP": AfP_d, "AbP": AbP_d,
        "CfO": CfO.astype(_BF16), "CbO": CbO.astype(_BF16),
        "lamf": lamf, "lamb": lamb,
        "bc1c": col(inputs["bc1"]), "bc2c": col(inputs["bc2"]),
        "bi_row": _bf(inputs["bi"]).reshape(1, D),
        "bo_row": _bf(inputs["bo"]).reshape(1, D),
        "g1v": _bf(inputs["g1"]),
        "b1v": _bf(inputs["b1"]),
        "g2v": _bf(inputs["g2"]),
    }
    in_maps = []
    b2_fold = np.asarray(inputs["b2"], f32)[None, :]
    for b in range(n_cores):
        xb = np.ascontiguousarray(x[b])
        m = dict(shared)
        m["x_res"] = xb + b2_fold
        m["xT"] = np.ascontiguousarray(xb.T.astype(_BF16))
        m["nsc"] = np.ascontiguousarray(ns[b].reshape(KT, P).T)
        in_maps.append(m)
    return in_maps


def get_nc(L=_L, D=_D, n_cores=_B, debug_taps=False):
    key = (L, D, n_cores)
    if key not in _cache:
        _cache[key] = _build(L, D, n_cores)
    return _cache[key]


def kernel(**inputs):
    from concourse.bass_utils import run_bass_kernel_spmd

    L, D, B = _L, _D, _B
    nc = get_nc(L, D, B)
    in_maps = _prep_maps(inputs, L, D, B)
    res = run_bass_kernel_spmd(nc, in_maps, core_ids=list(range(B)))
    return np.stack([res.results[c]["out"] for c in range(B)]).astype(
        np.float32)
